# revision 41
# baseline (speedup 1.0000x reference)
"""AttentionBlock (GroupNorm + single-head self-attention + residual) on 8 trn2 cores.

Sharding: core = (batch b = core//2, token-half h = core%2).  Each core gets the
full (128, 4096) channel-major image for its batch (needed for groupnorm stats
and full K/V), computes attention only for its 2048-token half, and writes a
(128, 2048) output slab.  The host rolls the token axis per-core so the q-half
is always columns [0:2048] -> one SPMD program for all 8 cores, no collectives.

Algebraic restructure (all exact, folded on the host):
  scoresT[m,n] = k_m . q_n = hn_m^T (Wk^T Wq') hn_n  (+ a per-n constant that
  cancels in softmax; Wq' absorbs the 1/sqrt(c) scale).  With M := Wk^T Wq'
  the score matmuls contract hn tiles against q2 = M.hn -- K is never
  materialized.  The k-bias shifts all scores of a q-column equally and drops
  out of softmax.  (Nonzero q-bias adds a per-m term: slow-path build flag.)

  attn out: y = x + Wp.(V.e)/den + bp,  V = Wv.hn + bv
          = x + (Wp.Wv).(hn.e)/den + (bp + Wp.bv)
  so the attn.V matmul uses plain PE *transposes* of hn tiles, and Wv/bv fold
  into the host-side proj weight W2 = Wp.Wv and bias bp2 = bp + Wp.bv.

Per core (c = channels on partitions):
  groupnorm stats via bn_stats + two tiny group-mask matmuls (cross-partition)
  hn = alpha*x + beta (DVE); q2 = M.hn (PE); hnT tiles via PE transpose
  per q-block (512) x k-triple (3x128):
      scoresT = hn_tile^T q2_blk  (PE, f32r, PSUM)
      attnT   = exp(scoresT)      (ACT, PSUM->SBUF, 1536-wide instr)
      out    += hnT_tile^T attnT  (PE, f32r, PSUM accumulate)
      den    += 1^T attnT         (PE ones-matmul / DVE adds, split 40/88)
  y = x + W2.(out) * (1/den broadcast) + bp2

float32r everywhere on the PE: full fp32 data, 1 cycle/row at N>=256
(plain float32 matmul streams at 1/4 rate on trn2).
"""

import numpy as np

C = 128        # channels
N = 4096       # tokens per batch (64*64)
NQ = 2048      # q tokens per core
B = 4
NCORES = 8
GROUPS = 8
EPS = 1e-5
QB = 512       # q block (one PSUM bank of fp32)
NQB = NQ // QB # 4
KT = 128       # k tile (partition dim)
NKT = N // KT  # 32
KT_GROUPS = [2] * 16   # k-tile pairs (exp granularity, 3-deep bufs)

_CACHE = {}


def _den_group_on_pe(gi):
    # 6 of 16 k-tile groups' denominator on PE, rest fused on DVE
    return gi % 8 < 3


def _build_nc(repeat=1, with_qbias=False):
    from contextlib import ExitStack

    import concourse.bacc as bacc
    import concourse.bass as bass
    import concourse.mybir as mybir
    import concourse.tile as tile
    from concourse.mybir import ActivationFunctionType as AF
    from concourse.mybir import AluOpType as ALU

    fp32 = mybir.dt.float32
    f32r = mybir.dt.float32r

    nc = bacc.Bacc()

    x_d = nc.dram_tensor("x", [C, N], fp32, kind="ExternalInput")
    mqk_d = nc.dram_tensor("mqk", [C, C], f32r, kind="ExternalInput")
    w2t_d = nc.dram_tensor("w2t", [C, C], f32r, kind="ExternalInput")
    ident_d = nc.dram_tensor("ident", [C, C], f32r, kind="ExternalInput")
    uq_d = nc.dram_tensor("uq", [C, 1], f32r, kind="ExternalInput")
    bp2_d = nc.dram_tensor("bp2", [C, 1], fp32, kind="ExternalInput")
    gsc_d = nc.dram_tensor("gscale", [C, 1], fp32, kind="ExternalInput")
    gbi_d = nc.dram_tensor("gbias", [C, 1], fp32, kind="ExternalInput")
    mka_d = nc.dram_tensor("maska", [C, GROUPS], fp32, kind="ExternalInput")
    mkb_d = nc.dram_tensor("maskb", [GROUPS, C], fp32, kind="ExternalInput")
    y_d = nc.dram_tensor("y", [C, NQ], fp32, kind="ExternalOutput")

    with tile.TileContext(nc) as tc, ExitStack() as ctx:
        const = ctx.enter_context(tc.tile_pool(name="const", bufs=1))
        big = ctx.enter_context(tc.tile_pool(name="big", bufs=1))
        small = ctx.enter_context(tc.tile_pool(name="small", bufs=1))

        # small consts on the sync queue (ahead of x), matrices on gpsimd's
        bp2_sb = const.tile([C, 1], fp32)
        nc.sync.dma_start(out=bp2_sb, in_=bp2_d[:, :])
        gsc_sb = const.tile([C, 1], fp32)
        nc.sync.dma_start(out=gsc_sb, in_=gsc_d[:, :])
        gbi_sb = const.tile([C, 1], fp32)
        nc.sync.dma_start(out=gbi_sb, in_=gbi_d[:, :])
        mka_sb = const.tile([C, GROUPS], fp32)
        nc.sync.dma_start(out=mka_sb, in_=mka_d[:, :])
        mkb_sb = const.tile([GROUPS, C], fp32)
        nc.sync.dma_start(out=mkb_sb, in_=mkb_d[:, :])
        uq_sb = const.tile([C, 1], f32r)
        nc.sync.dma_start(out=uq_sb, in_=uq_d[:, :])
        mqk_sb = const.tile([C, C], f32r)
        nc.gpsimd.dma_start(out=mqk_sb, in_=mqk_d[:, :])
        w2t_sb = const.tile([C, C], f32r)
        nc.gpsimd.dma_start(out=w2t_sb, in_=w2t_d[:, :])
        ident_sb = const.tile([C, C], f32r)
        nc.gpsimd.dma_start(out=ident_sb, in_=ident_d[:, :])

        ones_col = const.tile([C, 1], fp32)
        nc.vector.memset(ones_col, 1.0)
        ones_col_r = const.tile([C, 1], f32r)
        nc.vector.tensor_copy(ones_col_r, ones_col)
        ones_row_r = const.tile([1, QB], f32r)
        nc.vector.memset(ones_row_r.bitcast(mybir.dt.uint32), 0x3F800000)
        eps_sb = const.tile([C, 1], fp32)
        nc.vector.memset(eps_sb, EPS)
        warm_sb = const.tile([1, 1], fp32)
        nc.scalar.activation(warm_sb, eps_sb[0:1, :], AF.Exp, bias=0.0, scale=0.0)

        rep_ctx = tc.For_i(0, repeat, 1) if repeat > 1 else None
        if rep_ctx is not None:
            rep_ctx.__enter__()

        x_sb = big.tile([C, N], fp32, tag="x")
        for ci in range(8):
            eng = nc.sync if ci % 2 == 0 else nc.scalar
            eng.dma_start(
                out=x_sb[:, ci * 512 : (ci + 1) * 512],
                in_=x_d[:, ci * 512 : (ci + 1) * 512],
            )

        # ---- groupnorm stats ----
        NCHUNK = N // 512
        with tc.tile_pool(name="stat_ps", bufs=2, space="PSUM") as stat_ps:
            stats = small.tile([C, NCHUNK, 6], fp32)
            for i in range(NCHUNK):
                nc.vector.bn_stats(
                    out=stats[:, i, :], in_=x_sb[:, i * 512 : (i + 1) * 512]
                )
            mv = small.tile([C, 2], fp32)
            nc.vector.bn_aggr(out=mv, in_=stats)

            # S = [m, v + m^2] per channel
            S = small.tile([C, 2], fp32)
            nc.vector.tensor_copy(S[:, 0:1], mv[:, 0:1])
            msq = small.tile([C, 1], fp32)
            nc.vector.tensor_mul(msq, mv[:, 0:1], mv[:, 0:1])
            nc.vector.tensor_add(S[:, 1:2], mv[:, 1:2], msq)

            # group-reduce across partitions via mask matmuls
            g_ps = stat_ps.tile([GROUPS, 2], fp32)
            nc.tensor.matmul(g_ps, mka_sb, S, start=True, stop=True)
            g_sb = small.tile([GROUPS, 2], fp32)
            nc.vector.tensor_copy(g_sb, g_ps)
            g2_ps = stat_ps.tile([C, 2], fp32)
            nc.tensor.matmul(g2_ps, mkb_sb, g_sb, start=True, stop=True)

            gsz = C // GROUPS
            mean_g = small.tile([C, 1], fp32)
            nc.vector.tensor_scalar_mul(mean_g, g2_ps[:, 0:1], 1.0 / gsz)
            e2_g = small.tile([C, 1], fp32)
            nc.vector.tensor_scalar_mul(e2_g, g2_ps[:, 1:2], 1.0 / gsz)
            var_g = small.tile([C, 1], fp32)
            nc.vector.tensor_mul(var_g, mean_g, mean_g)
            nc.vector.tensor_tensor(out=var_g, in0=e2_g, in1=var_g, op=ALU.subtract)
            # rstd = exp(-0.5*ln(var+eps)) -- stays on the exp table set
            lnv_g = small.tile([C, 1], fp32)
            nc.scalar.activation(lnv_g, var_g, AF.Ln, bias=eps_sb, scale=1.0)
            rstd_g = small.tile([C, 1], fp32)
            nc.scalar.activation(rstd_g, lnv_g, AF.Exp, bias=0.0, scale=-0.5)
            alpha = small.tile([C, 1], fp32)
            nc.vector.tensor_mul(alpha, rstd_g, gsc_sb)
            beta = small.tile([C, 1], fp32)
            nc.vector.tensor_mul(beta, mean_g, alpha)
            nc.vector.tensor_tensor(out=beta, in0=gbi_sb, in1=beta, op=ALU.subtract)

        # ---- hn = alpha*x + beta (DVE, chunked) ----
        hn = big.tile([C, N], f32r, tag="hn")
        for ci in range(4):
            nc.vector.tensor_scalar(
                out=hn[:, ci * 1024 : (ci + 1) * 1024],
                in0=x_sb[:, ci * 1024 : (ci + 1) * 1024],
                scalar1=alpha, scalar2=beta, op0=ALU.mult, op1=ALU.add,
            )

        # ---- q2 = M.hn (q half), hnT tiles (full) ----
        q2_sb = big.tile([C, NQB, QB], f32r, tag="q2")
        hnT_sb = big.tile([KT, NKT, C], f32r, tag="hnT")
        gam_sb = None
        if with_qbias:
            gam_sb = big.tile([1, N], f32r, tag="gam")

        with (
            tc.tile_pool(name="qk_ps", bufs=2, space="PSUM") as qk_ps,
            tc.tile_pool(name="v_ps", bufs=2, space="PSUM") as v_ps,
        ):
            for j in range(NQB * 2):
                vp = v_ps.tile([KT, 4, C], fp32, tag="v")
                for t in range(4):
                    kt = j * 4 + t
                    nc.tensor.matmul(
                        vp[:, t, :], hn[:, kt * KT : (kt + 1) * KT], ident_sb,
                        start=True, stop=True,
                    )
                nc.vector.tensor_copy(hnT_sb[:, j * 4 : (j + 1) * 4, :], vp)
                if j >= NQB:
                    continue
                ps2 = qk_ps.tile([C, 512], fp32, tag="qk")
                nc.tensor.matmul(
                    ps2, mqk_sb, hn[:, j * 512 : (j + 1) * 512],
                    start=True, stop=True,
                )
                nc.vector.tensor_copy(q2_sb[:, j, :], ps2)
            if with_qbias:
                for j in range(N // 512):
                    gp = qk_ps.tile([1, 512], fp32, tag="qg")
                    nc.tensor.matmul(
                        gp, uq_sb, hn[:, j * 512 : (j + 1) * 512],
                        start=True, stop=True,
                    )
                    nc.vector.tensor_copy(
                        gam_sb[:, j * 512 : (j + 1) * 512], gp
                    )

        # ---- attention main loop ----
        aout_sb = big.tile([C, NQB, QB], f32r, tag="aout")
        rden_dram = nc.dram_tensor("rden_scratch", [NQB, QB], fp32, kind="Internal")
        with (
            tc.tile_pool(name="s_ps", bufs=3, space="PSUM") as spool,
            tc.tile_pool(name="o_ps", bufs=1, space="PSUM") as opool,
            tc.tile_pool(name="d_ps", bufs=1, space="PSUM") as dpool,
            tc.tile_pool(name="attn", bufs=3) as apool,
        ):
            def emit_y(pend):
                pp_sb_, rbc_, qb_ = pend
                y_sb = small.tile([C, QB], fp32, tag="y", bufs=2)
                nc.vector.tensor_mul(y_sb, pp_sb_, rbc_)
                nc.vector.tensor_add(
                    y_sb, y_sb, x_sb[:, qb_ * QB : (qb_ + 1) * QB]
                )
                nc.vector.tensor_scalar_add(y_sb, y_sb, bp2_sb)
                nc.sync.dma_start(out=y_d[:, qb_ * QB : (qb_ + 1) * QB], in_=y_sb)

            pending_y = None
            for qb in range(NQB):
                out_ps = opool.tile([C, QB], fp32, tag="out")
                den_ps = dpool.tile([1, QB], fp32, tag="den")
                den_sb = small.tile([KT, 2, QB], f32r, tag="densb", bufs=2)
                qv = q2_sb[:, qb, :]
                kt = 0
                dve_den_started = False
                pe_den_started = False
                for gi, gsize in enumerate(KT_GROUPS):
                    s_ps = spool.tile([KT, 2, QB], fp32, tag="s")
                    for t in range(gsize):
                        nc.tensor.matmul(
                            s_ps[:, t, :],
                            hn[:, (kt + t) * KT : (kt + t + 1) * KT],
                            qv,
                            start=True,
                            stop=(not with_qbias),
                        )
                        if with_qbias:
                            # += gamma[m] broadcast along q (ones row rhs)
                            nc.tensor.matmul(
                                s_ps[:, t, :],
                                gam_sb[:, (kt + t) * KT : (kt + t + 1) * KT],
                                ones_row_r,
                                start=False,
                                stop=True,
                            )
                    at = apool.tile([KT, 2, QB], f32r, tag="at")
                    nc.scalar.activation(at[:, :gsize, :], s_ps[:, :gsize, :], AF.Exp)
                    if gi == 2 and pending_y is not None:
                        emit_y(pending_y)
                        pending_y = None
                    for t in range(gsize):
                        k_idx = kt + t
                        nc.tensor.matmul(
                            out_ps,
                            hnT_sb[:, k_idx, :],
                            at[:, t, :],
                            start=(k_idx == 0),
                            stop=(k_idx == NKT - 1),
                        )
                        if _den_group_on_pe(gi):
                            nc.tensor.matmul(
                                den_ps,
                                ones_col_r,
                                at[:, t, :],
                                start=(not pe_den_started),
                                stop=False,
                            )
                            pe_den_started = True
                    if not _den_group_on_pe(gi):
                        # fused 1024-wide denominator accumulate on DVE
                        if not dve_den_started:
                            nc.vector.tensor_copy(
                                den_sb[:, :gsize, :], at[:, :gsize, :]
                            )
                            dve_den_started = True
                        else:
                            nc.vector.tensor_add(
                                den_sb[:, :gsize, :], den_sb[:, :gsize, :],
                                at[:, :gsize, :],
                            )
                    kt += gsize
                # fold the DVE part into den_ps (completes the accumulation)
                nc.tensor.matmul(den_ps, ones_col_r, den_sb[:, 0, :],
                                 start=False, stop=False)
                nc.tensor.matmul(den_ps, ones_col_r, den_sb[:, 1, :],
                                 start=False, stop=True)

                # copy unnormalized accumulator out early (frees out_ps without
                # waiting on the denominator chain)
                nc.vector.tensor_copy(aout_sb[:, qb, :], out_ps)

                # denominator chain: recip -> DRAM-bounce partition broadcast
                rden = small.tile([1, QB], fp32, tag="rden", bufs=2)
                nc.vector.reciprocal(rden, den_ps)
                nc.sync.dma_start(out=rden_dram[qb : qb + 1, :], in_=rden)
                rbc = small.tile([C, QB], fp32, tag="rbc", bufs=2)
                rd_ap = rden_dram[qb : qb + 1, :]
                nc.sync.dma_start(
                    out=rbc,
                    in_=bass.AP(
                        tensor=rd_ap.tensor, offset=rd_ap.offset, ap=[[0, C], [1, QB]]
                    ),
                )

                # proj on the unnormalized accumulator; the rbc-gated y chain is
                # deferred into the next q-block's pipeline (hides the DMA
                # broadcast latency and frees slots early)
                pp = opool.tile([C, QB], fp32, tag="out")
                nc.tensor.matmul(
                    pp, w2t_sb, aout_sb[:, qb, :], start=True, stop=True
                )
                pp_sb = small.tile([C, QB], fp32, tag="ppsb", bufs=2)
                nc.vector.tensor_copy(pp_sb, pp)
                pending_y = (pp_sb, rbc, qb)

            if pending_y is not None:
                emit_y(pending_y)

        if rep_ctx is not None:
            rep_ctx.__exit__(None, None, None)

    nc.compile()
    return nc


def _prep_maps(x):
    x = np.ascontiguousarray(np.asarray(x, dtype=np.float32))
    b, c, h, w = x.shape
    assert (b, c, h * w) == (B, C, N), f"unexpected shape {x.shape}"
    return x.reshape(b, c, h * w)


def _make_in_maps(x, norm_scale, norm_bias, wq, bq, wk, bk, wv, bv, wp, bp):
    xr = _prep_maps(x)
    s = float(C) ** -0.5
    f32 = np.float32
    f64 = np.float64

    wqs = np.asarray(wq, f64) * s
    wk64 = np.asarray(wk, f64)
    wv64 = np.asarray(wv, f64)
    wp64 = np.asarray(wp, f64)
    bq64 = np.asarray(bq, f64) * s
    bv64 = np.asarray(bv, f64)
    bp64 = np.asarray(bp, f64)

    # scores: hn^T (Wk^T Wq') hn ; lhsT for q2 = M.hn is M^T = Wq'^T Wk
    mqk = np.ascontiguousarray((wqs.T @ wk64).astype(f32))
    # proj: W2 = Wp.Wv, lhsT = W2^T ; bias bp2 = bp + Wp.bv
    w2t = np.ascontiguousarray((wp64 @ wv64).T.astype(f32))
    bp2 = np.ascontiguousarray((bp64 + wp64 @ bv64).astype(f32).reshape(C, 1))
    # q-bias term (slow path only): u = Wk^T bq'
    uq = np.ascontiguousarray((wk64.T @ bq64).astype(f32).reshape(C, 1))
    ident = np.ascontiguousarray(np.eye(C, dtype=f32))
    gsc = np.ascontiguousarray(np.asarray(norm_scale, f32).reshape(C, 1))
    gbi = np.ascontiguousarray(np.asarray(norm_bias, f32).reshape(C, 1))
    maska = np.zeros((C, GROUPS), f32)
    maska[np.arange(C), np.arange(C) // (C // GROUPS)] = 1.0
    maskb = np.ascontiguousarray(maska.T)

    with_qbias = bool(np.any(np.asarray(bq) != 0))

    in_maps = []
    for core in range(NCORES):
        bi, hi = core // 2, core % 2
        xb = xr[bi]
        if hi:
            xb = np.roll(xb, -NQ, axis=1)
        in_maps.append(
            dict(
                x=np.ascontiguousarray(xb),
                mqk=mqk, w2t=w2t, ident=ident, uq=uq, bp2=bp2,
                gscale=gsc, gbias=gbi, maska=maska, maskb=maskb,
            )
        )
    return in_maps, with_qbias


def kernel(x, norm_scale, norm_bias, wq, bq, wk, bk, wv, bv, wp, bp):
    from concourse.bass_utils import run_bass_kernel_spmd

    in_maps, with_qbias = _make_in_maps(
        x, norm_scale, norm_bias, wq, bq, wk, bk, wv, bv, wp, bp
    )

    key = ("nc", with_qbias)
    if key not in _CACHE:
        _CACHE[key] = _build_nc(with_qbias=with_qbias)
    res = run_bass_kernel_spmd(
        _CACHE[key], in_maps, core_ids=list(range(NCORES)), **_CACHE.get("runkw", {})
    )
    _CACHE["last_result"] = res

    out = np.empty((B, C, N), np.float32)
    for core in range(NCORES):
        bi, hi = core // 2, core % 2
        out[bi, :, hi * NQ : (hi + 1) * NQ] = res.results[core]["y"]
    return out.reshape(B, C, 64, 64)


# revision 42
# speedup vs baseline: 1.1231x; 1.1231x over previous
"""AttentionBlock (GroupNorm + single-head self-attention + residual) on 8 trn2 cores.

Sharding: core = (batch b = core//2, token-half h = core%2).  Each core gets the
full (128, 4096) channel-major image for its batch (needed for groupnorm stats
and full K/V), computes attention only for its 2048-token half, and writes a
(128, 2048) output slab.  The host rolls the token axis per-core so the q-half
is always columns [0:2048] -> one SPMD program for all 8 cores, no collectives.

Algebraic restructure (all exact, folded on the host):
  scoresT[m,n] = k_m . q_n = hn_m^T (Wk^T Wq') hn_n  (+ a per-n constant that
  cancels in softmax; Wq' absorbs the 1/sqrt(c) scale).  With M := Wk^T Wq'
  the score matmuls contract hn tiles against q2 = M.hn -- K is never
  materialized.  The k-bias shifts all scores of a q-column equally and drops
  out of softmax.  (Nonzero q-bias adds a per-m term: slow-path build flag.)

  attn out: y = x + Wp.(V.e)/den + bp,  V = Wv.hn + bv
          = x + (Wp.Wv).(hn.e)/den + (bp + Wp.bv)
  so the attn.V matmul uses plain PE *transposes* of hn tiles, and Wv/bv fold
  into the host-side proj weight W2 = Wp.Wv and bias bp2 = bp + Wp.bv.

Per core (c = channels on partitions):
  groupnorm stats via bn_stats + two tiny group-mask matmuls (cross-partition)
  hn = alpha*x + beta (DVE); q2 = M.hn (PE); hnT tiles via PE transpose
  per q-block (512) x k-triple (3x128):
      scoresT = hn_tile^T q2_blk  (PE, f32r, PSUM)
      attnT   = exp(scoresT)      (ACT, PSUM->SBUF, 1536-wide instr)
      out    += hnT_tile^T attnT  (PE, f32r, PSUM accumulate)
      den    += 1^T attnT         (PE ones-matmul / DVE adds, split 40/88)
  y = x + W2.(out) * (1/den broadcast) + bp2

float32r everywhere on the PE: full fp32 data, 1 cycle/row at N>=256
(plain float32 matmul streams at 1/4 rate on trn2).
"""

import numpy as np

C = 128        # channels
N = 4096       # tokens per batch (64*64)
NQ = 2048      # q tokens per core
B = 4
NCORES = 8
GROUPS = 8
EPS = 1e-5
QB = 512       # q block (one PSUM bank of fp32)
NQB = NQ // QB # 4
KT = 128       # k tile (partition dim)
NKT = N // KT  # 32
KT_GROUPS = [3] * 10 + [2]   # k-tile triples (fewer ACT overheads)

_CACHE = {}


def _den_group_on_pe(gi):
    # 4 of 11 k-tile groups' denominator on PE, rest fused on DVE
    return gi % 3 == 0


def _build_nc(repeat=1, with_qbias=False):
    from contextlib import ExitStack

    import concourse.bacc as bacc
    import concourse.bass as bass
    import concourse.mybir as mybir
    import concourse.tile as tile
    from concourse.mybir import ActivationFunctionType as AF
    from concourse.mybir import AluOpType as ALU

    fp32 = mybir.dt.float32
    f32r = mybir.dt.float32r

    nc = bacc.Bacc()

    x_d = nc.dram_tensor("x", [C, N], fp32, kind="ExternalInput")
    mqk_d = nc.dram_tensor("mqk", [C, C], f32r, kind="ExternalInput")
    w2t_d = nc.dram_tensor("w2t", [C, C], f32r, kind="ExternalInput")
    ident_d = nc.dram_tensor("ident", [C, C], f32r, kind="ExternalInput")
    uq_d = nc.dram_tensor("uq", [C, 1], f32r, kind="ExternalInput")
    bp2_d = nc.dram_tensor("bp2", [C, 1], fp32, kind="ExternalInput")
    gsc_d = nc.dram_tensor("gscale", [C, 1], fp32, kind="ExternalInput")
    gbi_d = nc.dram_tensor("gbias", [C, 1], fp32, kind="ExternalInput")
    mka_d = nc.dram_tensor("maska", [C, GROUPS], fp32, kind="ExternalInput")
    mkb_d = nc.dram_tensor("maskb", [GROUPS, C], fp32, kind="ExternalInput")
    y_d = nc.dram_tensor("y", [C, NQ], fp32, kind="ExternalOutput")

    with tile.TileContext(nc) as tc, ExitStack() as ctx:
        const = ctx.enter_context(tc.tile_pool(name="const", bufs=1))
        big = ctx.enter_context(tc.tile_pool(name="big", bufs=1))
        small = ctx.enter_context(tc.tile_pool(name="small", bufs=1))

        # small consts on the sync queue (ahead of x), matrices on gpsimd's
        bp2_sb = const.tile([C, 1], fp32)
        nc.sync.dma_start(out=bp2_sb, in_=bp2_d[:, :])
        gsc_sb = const.tile([C, 1], fp32)
        nc.sync.dma_start(out=gsc_sb, in_=gsc_d[:, :])
        gbi_sb = const.tile([C, 1], fp32)
        nc.sync.dma_start(out=gbi_sb, in_=gbi_d[:, :])
        mka_sb = const.tile([C, GROUPS], fp32)
        nc.sync.dma_start(out=mka_sb, in_=mka_d[:, :])
        mkb_sb = const.tile([GROUPS, C], fp32)
        nc.sync.dma_start(out=mkb_sb, in_=mkb_d[:, :])
        uq_sb = const.tile([C, 1], f32r)
        nc.sync.dma_start(out=uq_sb, in_=uq_d[:, :])
        mqk_sb = const.tile([C, C], f32r)
        nc.gpsimd.dma_start(out=mqk_sb, in_=mqk_d[:, :])
        w2t_sb = const.tile([C, C], f32r)
        nc.gpsimd.dma_start(out=w2t_sb, in_=w2t_d[:, :])
        ident_sb = const.tile([C, C], f32r)
        nc.gpsimd.dma_start(out=ident_sb, in_=ident_d[:, :])

        ones_col = const.tile([C, 1], fp32)
        nc.vector.memset(ones_col, 1.0)
        ones_col_r = const.tile([C, 1], f32r)
        nc.vector.tensor_copy(ones_col_r, ones_col)
        ones_row_r = const.tile([1, QB], f32r)
        nc.vector.memset(ones_row_r.bitcast(mybir.dt.uint32), 0x3F800000)
        eps_sb = const.tile([C, 1], fp32)
        nc.vector.memset(eps_sb, EPS)
        warm_sb = const.tile([1, 1], fp32)
        nc.scalar.activation(warm_sb, eps_sb[0:1, :], AF.Exp, bias=0.0, scale=0.0)

        rep_ctx = tc.For_i(0, repeat, 1) if repeat > 1 else None
        if rep_ctx is not None:
            rep_ctx.__enter__()

        x_sb = big.tile([C, N], fp32, tag="x")
        for ci in range(8):
            eng = nc.sync if ci % 2 == 0 else nc.scalar
            eng.dma_start(
                out=x_sb[:, ci * 512 : (ci + 1) * 512],
                in_=x_d[:, ci * 512 : (ci + 1) * 512],
            )

        # ---- groupnorm stats ----
        NCHUNK = N // 512
        with tc.tile_pool(name="stat_ps", bufs=2, space="PSUM") as stat_ps:
            stats = small.tile([C, NCHUNK, 6], fp32)
            for i in range(NCHUNK):
                nc.vector.bn_stats(
                    out=stats[:, i, :], in_=x_sb[:, i * 512 : (i + 1) * 512]
                )
            mv = small.tile([C, 2], fp32)
            nc.vector.bn_aggr(out=mv, in_=stats)

            # S = [m, v + m^2] per channel
            S = small.tile([C, 2], fp32)
            nc.vector.tensor_copy(S[:, 0:1], mv[:, 0:1])
            msq = small.tile([C, 1], fp32)
            nc.vector.tensor_mul(msq, mv[:, 0:1], mv[:, 0:1])
            nc.vector.tensor_add(S[:, 1:2], mv[:, 1:2], msq)

            # group-reduce across partitions via mask matmuls
            g_ps = stat_ps.tile([GROUPS, 2], fp32)
            nc.tensor.matmul(g_ps, mka_sb, S, start=True, stop=True)
            g_sb = small.tile([GROUPS, 2], fp32)
            nc.vector.tensor_copy(g_sb, g_ps)
            g2_ps = stat_ps.tile([C, 2], fp32)
            nc.tensor.matmul(g2_ps, mkb_sb, g_sb, start=True, stop=True)

            gsz = C // GROUPS
            mean_g = small.tile([C, 1], fp32)
            nc.vector.tensor_scalar_mul(mean_g, g2_ps[:, 0:1], 1.0 / gsz)
            e2_g = small.tile([C, 1], fp32)
            nc.vector.tensor_scalar_mul(e2_g, g2_ps[:, 1:2], 1.0 / gsz)
            var_g = small.tile([C, 1], fp32)
            nc.vector.tensor_mul(var_g, mean_g, mean_g)
            nc.vector.tensor_tensor(out=var_g, in0=e2_g, in1=var_g, op=ALU.subtract)
            # rstd = exp(-0.5*ln(var+eps)) -- stays on the exp table set
            lnv_g = small.tile([C, 1], fp32)
            nc.scalar.activation(lnv_g, var_g, AF.Ln, bias=eps_sb, scale=1.0)
            rstd_g = small.tile([C, 1], fp32)
            nc.scalar.activation(rstd_g, lnv_g, AF.Exp, bias=0.0, scale=-0.5)
            alpha = small.tile([C, 1], fp32)
            nc.vector.tensor_mul(alpha, rstd_g, gsc_sb)
            beta = small.tile([C, 1], fp32)
            nc.vector.tensor_mul(beta, mean_g, alpha)
            nc.vector.tensor_tensor(out=beta, in0=gbi_sb, in1=beta, op=ALU.subtract)

        # ---- hn = alpha*x + beta (DVE, chunked) ----
        hn = big.tile([C, N], f32r, tag="hn")
        for ci in range(4):
            nc.vector.tensor_scalar(
                out=hn[:, ci * 1024 : (ci + 1) * 1024],
                in0=x_sb[:, ci * 1024 : (ci + 1) * 1024],
                scalar1=alpha, scalar2=beta, op0=ALU.mult, op1=ALU.add,
            )

        # ---- q2 = M.hn (q half), hnT tiles (full) ----
        q2_sb = big.tile([C, NQB, QB], f32r, tag="q2")
        hnT_sb = big.tile([KT, NKT, C], f32r, tag="hnT")
        gam_sb = None
        if with_qbias:
            gam_sb = big.tile([1, N], f32r, tag="gam")

        with (
            tc.tile_pool(name="qk_ps", bufs=2, space="PSUM") as qk_ps,
            tc.tile_pool(name="v_ps", bufs=2, space="PSUM") as v_ps,
        ):
            for j in range(NQB * 2):
                vp = v_ps.tile([KT, 4, C], fp32, tag="v")
                for t in range(4):
                    kt = j * 4 + t
                    nc.tensor.matmul(
                        vp[:, t, :], hn[:, kt * KT : (kt + 1) * KT], ident_sb,
                        start=True, stop=True,
                    )
                nc.vector.tensor_copy(hnT_sb[:, j * 4 : (j + 1) * 4, :], vp)
                if j >= NQB:
                    continue
                ps2 = qk_ps.tile([C, 512], fp32, tag="qk")
                nc.tensor.matmul(
                    ps2, mqk_sb, hn[:, j * 512 : (j + 1) * 512],
                    start=True, stop=True,
                )
                nc.vector.tensor_copy(q2_sb[:, j, :], ps2)
            if with_qbias:
                for j in range(N // 512):
                    gp = qk_ps.tile([1, 512], fp32, tag="qg")
                    nc.tensor.matmul(
                        gp, uq_sb, hn[:, j * 512 : (j + 1) * 512],
                        start=True, stop=True,
                    )
                    nc.vector.tensor_copy(
                        gam_sb[:, j * 512 : (j + 1) * 512], gp
                    )

        # ---- attention main loop ----
        aout_sb = big.tile([C, NQB, QB], f32r, tag="aout")
        rden_dram = nc.dram_tensor("rden_scratch", [NQB, QB], fp32, kind="Internal")
        with (
            tc.tile_pool(name="s_ps", bufs=2, space="PSUM") as spool,
            tc.tile_pool(name="o_ps", bufs=1, space="PSUM") as opool,
            tc.tile_pool(name="d_ps", bufs=1, space="PSUM") as dpool,
            tc.tile_pool(name="attn", bufs=3) as apool,
        ):
            def emit_y(pend):
                pp_sb_, rbc_, qb_ = pend
                y_sb = small.tile([C, QB], fp32, tag="y", bufs=2)
                nc.vector.tensor_mul(y_sb, pp_sb_, rbc_)
                nc.vector.tensor_add(
                    y_sb, y_sb, x_sb[:, qb_ * QB : (qb_ + 1) * QB]
                )
                nc.vector.tensor_scalar_add(y_sb, y_sb, bp2_sb)
                nc.sync.dma_start(out=y_d[:, qb_ * QB : (qb_ + 1) * QB], in_=y_sb)

            pending_y = None
            for qb in range(NQB):
                out_ps = opool.tile([C, QB], fp32, tag="out")
                den_ps = dpool.tile([1, QB], fp32, tag="den")
                den_sb = small.tile([KT, 3, QB], f32r, tag="densb", bufs=2)
                qv = q2_sb[:, qb, :]
                kt = 0
                dve_den_started = False
                pe_den_started = False
                for gi, gsize in enumerate(KT_GROUPS):
                    s_ps = spool.tile([KT, 3, QB], fp32, tag="s")
                    for t in range(gsize):
                        nc.tensor.matmul(
                            s_ps[:, t, :],
                            hn[:, (kt + t) * KT : (kt + t + 1) * KT],
                            qv,
                            start=True,
                            stop=(not with_qbias),
                        )
                        if with_qbias:
                            # += gamma[m] broadcast along q (ones row rhs)
                            nc.tensor.matmul(
                                s_ps[:, t, :],
                                gam_sb[:, (kt + t) * KT : (kt + t + 1) * KT],
                                ones_row_r,
                                start=False,
                                stop=True,
                            )
                    at = apool.tile([KT, 3, QB], f32r, tag="at")
                    nc.scalar.activation(at[:, :gsize, :], s_ps[:, :gsize, :], AF.Exp)
                    if gi == 2 and pending_y is not None:
                        emit_y(pending_y)
                        pending_y = None
                    for t in range(gsize):
                        k_idx = kt + t
                        nc.tensor.matmul(
                            out_ps,
                            hnT_sb[:, k_idx, :],
                            at[:, t, :],
                            start=(k_idx == 0),
                            stop=(k_idx == NKT - 1),
                        )
                        if _den_group_on_pe(gi):
                            nc.tensor.matmul(
                                den_ps,
                                ones_col_r,
                                at[:, t, :],
                                start=(not pe_den_started),
                                stop=False,
                            )
                            pe_den_started = True
                    if not _den_group_on_pe(gi):
                        # fused 1024-wide denominator accumulate on DVE
                        if not dve_den_started:
                            nc.vector.tensor_copy(
                                den_sb[:, :gsize, :], at[:, :gsize, :]
                            )
                            dve_den_started = True
                        else:
                            nc.vector.tensor_add(
                                den_sb[:, :gsize, :], den_sb[:, :gsize, :],
                                at[:, :gsize, :],
                            )
                    kt += gsize
                # fold the DVE part into den_ps (completes the accumulation)
                nc.tensor.matmul(den_ps, ones_col_r, den_sb[:, 0, :],
                                 start=False, stop=False)
                nc.tensor.matmul(den_ps, ones_col_r, den_sb[:, 1, :],
                                 start=False, stop=False)
                nc.tensor.matmul(den_ps, ones_col_r, den_sb[:, 2, :],
                                 start=False, stop=True)

                # copy unnormalized accumulator out early (frees out_ps without
                # waiting on the denominator chain)
                nc.vector.tensor_copy(aout_sb[:, qb, :], out_ps)

                # denominator chain: recip -> DRAM-bounce partition broadcast
                rden = small.tile([1, QB], fp32, tag="rden", bufs=2)
                nc.vector.reciprocal(rden, den_ps)
                nc.sync.dma_start(out=rden_dram[qb : qb + 1, :], in_=rden)
                rbc = small.tile([C, QB], fp32, tag="rbc", bufs=2)
                rd_ap = rden_dram[qb : qb + 1, :]
                nc.sync.dma_start(
                    out=rbc,
                    in_=bass.AP(
                        tensor=rd_ap.tensor, offset=rd_ap.offset, ap=[[0, C], [1, QB]]
                    ),
                )

                # proj on the unnormalized accumulator; the rbc-gated y chain is
                # deferred into the next q-block's pipeline (hides the DMA
                # broadcast latency and frees slots early)
                pp = opool.tile([C, QB], fp32, tag="out")
                nc.tensor.matmul(
                    pp, w2t_sb, aout_sb[:, qb, :], start=True, stop=True
                )
                pp_sb = small.tile([C, QB], fp32, tag="ppsb", bufs=2)
                nc.vector.tensor_copy(pp_sb, pp)
                pending_y = (pp_sb, rbc, qb)

            if pending_y is not None:
                emit_y(pending_y)

        if rep_ctx is not None:
            rep_ctx.__exit__(None, None, None)

    nc.compile()
    return nc


def _prep_maps(x):
    x = np.ascontiguousarray(np.asarray(x, dtype=np.float32))
    b, c, h, w = x.shape
    assert (b, c, h * w) == (B, C, N), f"unexpected shape {x.shape}"
    return x.reshape(b, c, h * w)


def _make_in_maps(x, norm_scale, norm_bias, wq, bq, wk, bk, wv, bv, wp, bp):
    xr = _prep_maps(x)
    s = float(C) ** -0.5
    f32 = np.float32
    f64 = np.float64

    wqs = np.asarray(wq, f64) * s
    wk64 = np.asarray(wk, f64)
    wv64 = np.asarray(wv, f64)
    wp64 = np.asarray(wp, f64)
    bq64 = np.asarray(bq, f64) * s
    bv64 = np.asarray(bv, f64)
    bp64 = np.asarray(bp, f64)

    # scores: hn^T (Wk^T Wq') hn ; lhsT for q2 = M.hn is M^T = Wq'^T Wk
    mqk = np.ascontiguousarray((wqs.T @ wk64).astype(f32))
    # proj: W2 = Wp.Wv, lhsT = W2^T ; bias bp2 = bp + Wp.bv
    w2t = np.ascontiguousarray((wp64 @ wv64).T.astype(f32))
    bp2 = np.ascontiguousarray((bp64 + wp64 @ bv64).astype(f32).reshape(C, 1))
    # q-bias term (slow path only): u = Wk^T bq'
    uq = np.ascontiguousarray((wk64.T @ bq64).astype(f32).reshape(C, 1))
    ident = np.ascontiguousarray(np.eye(C, dtype=f32))
    gsc = np.ascontiguousarray(np.asarray(norm_scale, f32).reshape(C, 1))
    gbi = np.ascontiguousarray(np.asarray(norm_bias, f32).reshape(C, 1))
    maska = np.zeros((C, GROUPS), f32)
    maska[np.arange(C), np.arange(C) // (C // GROUPS)] = 1.0
    maskb = np.ascontiguousarray(maska.T)

    with_qbias = bool(np.any(np.asarray(bq) != 0))

    in_maps = []
    for core in range(NCORES):
        bi, hi = core // 2, core % 2
        xb = xr[bi]
        if hi:
            xb = np.roll(xb, -NQ, axis=1)
        in_maps.append(
            dict(
                x=np.ascontiguousarray(xb),
                mqk=mqk, w2t=w2t, ident=ident, uq=uq, bp2=bp2,
                gscale=gsc, gbias=gbi, maska=maska, maskb=maskb,
            )
        )
    return in_maps, with_qbias


def kernel(x, norm_scale, norm_bias, wq, bq, wk, bk, wv, bv, wp, bp):
    from concourse.bass_utils import run_bass_kernel_spmd

    in_maps, with_qbias = _make_in_maps(
        x, norm_scale, norm_bias, wq, bq, wk, bk, wv, bv, wp, bp
    )

    key = ("nc", with_qbias)
    if key not in _CACHE:
        _CACHE[key] = _build_nc(with_qbias=with_qbias)
    res = run_bass_kernel_spmd(
        _CACHE[key], in_maps, core_ids=list(range(NCORES)), **_CACHE.get("runkw", {})
    )
    _CACHE["last_result"] = res

    out = np.empty((B, C, N), np.float32)
    for core in range(NCORES):
        bi, hi = core // 2, core % 2
        out[bi, :, hi * NQ : (hi + 1) * NQ] = res.results[core]["y"]
    return out.reshape(B, C, 64, 64)
